# revision 5
# baseline (speedup 1.0000x reference)
"""3D bilateral filter (window 3, sigma_d=120, sigma_r=1.2) on 8 TRN2 NeuronCores.

Algorithm: factor the range kernel
    exp(-(n-c)^2/a) = phi(n) * phi(c) * exp(2*n*c/a),   phi(x) = exp(-x^2/a)
and approximate exp(2*t/a) on t in [0,1] by a degree-J polynomial
    exp(2t/a) ~= sum_j p_j t^j.
Then with moment fields  phi_j = phi(v) * v^j  and  G_j = conv3x3x3(s, phi_j)
(s = separable spatial Gaussian [alpha,1,alpha] per axis):
    den = phi(c) * sum_j p_j c^j G_j
    num = phi(c) * sum_j p_j c^j G_{j+1}
    out = num / den            (phi(c) cancels)
The 3D conv runs on the Tensor engine: the D-axis (partition dim) conv is a
banded 128x128 matmul (replicate edges folded into the corner entries), and
the 9 (dh,dw) shifts are free-dim AP offsets accumulated in PSUM.

Sharding: 8 cores split H (192 -> 24 rows each) with 1-row halo overlap,
prepared host-side. No cross-core communication.
"""

import sys

for _p in ("/opt/trn_rl_repo",):
    if _p not in sys.path:
        sys.path.insert(0, _p)

import numpy as np

# ---------------- problem constants (hardcoded per spec) ----------------
B, D, H, W = 2, 128, 192, 192
SIGMA_D = 120.0
SIGMA_R = 1.2
A = 2.0 * SIGMA_R * SIGMA_R                 # 2.88
ALPHA = float(np.exp(-1.0 / (2.0 * SIGMA_D * SIGMA_D)))

N_CORES = 8
HPC = H // N_CORES                          # 24 output rows per core
WW = W + 2                                  # W padded with replicate halo

# tunables
J = 5                                       # polynomial degree for exp(2t/a)
NMOM = J + 2                                # moments G_0..G_{J+1}
CH = 4                                      # output rows per chunk
NCH = HPC // CH
SUBROWS = 2                                 # rows per PSUM sub-chunk (<=512 fp32 bank)
NSUB = CH // SUBROWS
MM_DTYPE = "float32r"                       # conv matmul dtype: float32r|float32|bfloat16


def _fit_poly(deg):
    # least-squares fit of exp(2t/A) at Chebyshev nodes on [0,1]
    t = (np.cos(np.pi * (np.arange(4000) + 0.5) / 4000) + 1.0) / 2.0
    y = np.exp(2.0 * t / A)
    V = np.vander(t, deg + 1, increasing=True)
    p, *_ = np.linalg.lstsq(V, y, rcond=None)
    return [float(c) for c in p]


PCOEF = _fit_poly(J)


def _band_matrices():
    """D-axis conv band matrix (with replicate-edge corners) x 3 spatial scales."""
    b0 = np.zeros((128, 128), np.float64)
    for i in range(128):
        b0[i, i] = 1.0
        if i > 0:
            b0[i - 1, i] = ALPHA
        if i < 127:
            b0[i + 1, i] = ALPHA
    b0[0, 0] += ALPHA
    b0[127, 127] += ALPHA
    bands = np.concatenate(
        [b0, ALPHA * b0, (ALPHA * ALPHA) * b0], axis=1
    )  # [128, 384]
    return bands.astype(np.float32)


_COMPILED = None


def _build():
    import concourse.bacc as bacc
    import concourse.mybir as mybir
    import concourse.tile as tile

    f32 = mybir.dt.float32
    mmdt = getattr(mybir.dt, MM_DTYPE)
    AF = mybir.ActivationFunctionType
    OP = mybir.AluOpType

    nc = bacc.Bacc("TRN2", target_bir_lowering=False, debug=False)
    vol = nc.dram_tensor("vol", [B, D, HPC + 2, WW], f32, kind="ExternalInput")
    bands = nc.dram_tensor("bands", [128, 3 * 128], f32, kind="ExternalInput")
    out = nc.dram_tensor("out", [B, D, HPC, W], f32, kind="ExternalOutput")

    HR = CH + 2                     # halo rows per chunk
    FHALO = HR * WW                 # free size of halo-extent tiles
    FOUT = CH * W                   # free size of output-extent tiles
    FSUB = SUBROWS * W              # free size of one PSUM sub-chunk

    with tile.TileContext(nc) as tc:
        with tc.tile_pool(name="const", bufs=1) as cpool, \
             tc.tile_pool(name="sbuf", bufs=2) as pool, \
             tc.tile_pool(name="gpool", bufs=2) as gpool, \
             tc.tile_pool(name="hpool", bufs=1) as hpool, \
             tc.tile_pool(name="psum", bufs=8, space="PSUM") as psum:

            bf = cpool.tile([128, 3 * 128], f32, tag="bands_f32")
            nc.sync.dma_start(bf[:, :], bands.ap())
            bmm = cpool.tile([128, 3 * 128], mmdt, tag="bands_mm")
            nc.vector.tensor_copy(bmm[:, :], bf[:, :])
            bmats = [bmm[:, 128 * m:128 * (m + 1)] for m in range(3)]

            # (dh, dw) -> band matrix index by dh^2+dw^2
            offsets = [(dh, dw) for dh in (-1, 0, 1) for dw in (-1, 0, 1)]

            for b in range(B):
                for ich in range(NCH):
                    r0 = ich * CH          # first output row of chunk (slab row r0+1)
                    vch = pool.tile([128, FHALO], f32, tag="vch")
                    nc.sync.dma_start(
                        vch[:, :], vol.ap()[b, :, r0:r0 + HR, :])
                    vchv = vch[:, :].rearrange("p (r w) -> p r w", r=HR)

                    phis = []
                    ph0 = pool.tile([128, FHALO], mmdt, tag="phi0")
                    nc.scalar.activation(ph0[:, :], vch[:, :], AF.Square)
                    nc.scalar.activation(ph0[:, :], ph0[:, :], AF.Exp,
                                         scale=-1.0 / A)
                    phis.append(ph0)
                    for j in range(1, NMOM):
                        pj = pool.tile([128, FHALO], mmdt, tag=f"phi{j}")
                        nc.vector.tensor_tensor(
                            pj[:, :], phis[-1][:, :], vch[:, :], op=OP.mult)
                        phis.append(pj)

                    gt = [gpool.tile([128, FOUT], f32, tag=f"G{j}",
                                     name=f"G{j}_{b}_{ich}")
                          for j in range(NMOM)]
                    for isub in range(NSUB):
                        rr = isub * SUBROWS            # output row in chunk
                        for j in range(NMOM):
                            ps = psum.tile([128, FSUB], f32, tag="ps")
                            pv = phis[j][:, :].rearrange(
                                "p (r w) -> p r w", r=HR)
                            for k, (dh, dw) in enumerate(offsets):
                                m = dh * dh + dw * dw
                                rhs = pv[:, rr + 1 + dh: rr + 1 + dh + SUBROWS,
                                         dw + 1: dw + 1 + W]
                                nc.tensor.matmul(
                                    ps[:, :], bmats[m], rhs,
                                    start=(k == 0), stop=(k == len(offsets) - 1))
                            nc.scalar.copy(
                                gt[j][:, rr * W:(rr + SUBROWS) * W], ps[:, :])

                    # center values c: interior of vch
                    cap = vchv[:, 1:CH + 1, 1:W + 1]

                    hn = hpool.tile([128, FOUT], f32, tag="hn")
                    hd = hpool.tile([128, FOUT], f32, tag="hd")
                    tn = hpool.tile([128, FOUT], f32, tag="tn")
                    td = hpool.tile([128, FOUT], f32, tag="td")
                    nc.vector.tensor_scalar_mul(hd[:, :], gt[J][:, :], PCOEF[J])
                    nc.vector.tensor_scalar_mul(hn[:, :], gt[J + 1][:, :], PCOEF[J])
                    for j in range(J - 1, -1, -1):
                        nc.vector.tensor_tensor(td[:, :], hd[:, :], cap, op=OP.mult)
                        nc.vector.scalar_tensor_tensor(
                            hd[:, :], gt[j][:, :], PCOEF[j], td[:, :],
                            op0=OP.mult, op1=OP.add)
                        nc.vector.tensor_tensor(tn[:, :], hn[:, :], cap, op=OP.mult)
                        nc.vector.scalar_tensor_tensor(
                            hn[:, :], gt[j + 1][:, :], PCOEF[j], tn[:, :],
                            op0=OP.mult, op1=OP.add)

                    # out = hn / hd
                    nc.vector.reciprocal_approx_accurate(
                        out=td[:, :], in_=hd[:, :], scratch=tn[:, :])
                    ot = pool.tile([128, FOUT], f32, tag="ot")
                    nc.vector.tensor_tensor(ot[:, :], hn[:, :], td[:, :], op=OP.mult)

                    nc.sync.dma_start(out.ap()[b, :, r0:r0 + CH, :], ot[:, :])

    nc.compile()
    return nc


def _get_compiled():
    global _COMPILED
    if _COMPILED is None:
        _COMPILED = _build()
    return _COMPILED


def _shard_inputs(volume):
    v = np.asarray(volume)[:, 0]                          # (B, D, H, W)
    vp = np.pad(v, ((0, 0), (0, 0), (1, 1), (1, 1)), mode="edge")
    bands = _band_matrices()
    in_maps = []
    for c in range(N_CORES):
        slab = np.ascontiguousarray(vp[:, :, c * HPC:c * HPC + HPC + 2, :])
        in_maps.append({"vol": slab, "bands": bands})
    return in_maps


def _run(volume, trace=False):
    from concourse import bass_utils
    nc = _get_compiled()
    in_maps = _shard_inputs(volume)
    res = bass_utils.run_bass_kernel_spmd(
        nc, in_maps, core_ids=list(range(N_CORES)), trace=trace)
    shards = [res.results[c]["out"] for c in range(N_CORES)]
    full = np.concatenate(shards, axis=2)                 # (B, D, H, W)
    return full[:, None].astype(np.float32), res


def kernel(volume):
    out, _ = _run(volume, trace=False)
    return out


# revision 6
# speedup vs baseline: 1.1985x; 1.1985x over previous
"""3D bilateral filter (window 3, sigma_d=120, sigma_r=1.2) on 8 TRN2 NeuronCores.

Algorithm: factor the range kernel
    exp(-(n-c)^2/a) = phi(n) * phi(c) * exp(2*n*c/a),   phi(x) = exp(-x^2/a)
and approximate exp(2*t/a) on t in [0,1] by a degree-J polynomial
    exp(2t/a) ~= sum_j p_j t^j.
Then with moment fields  phi_j = phi(v) * v^j  and  G_j = conv3x3x3(s, phi_j)
(s = separable spatial Gaussian [alpha,1,alpha] per axis):
    den = phi(c) * sum_j p_j c^j G_j
    num = phi(c) * sum_j p_j c^j G_{j+1}
    out = num / den            (phi(c) cancels)
The 3D conv runs on the Tensor engine: the D-axis (partition dim) conv is a
banded 128x128 matmul (replicate edges folded into the corner entries), and
the 9 (dh,dw) shifts are free-dim AP offsets accumulated in PSUM.

Sharding: 8 cores split H (192 -> 24 rows each) with 1-row halo overlap,
prepared host-side. No cross-core communication.
"""

import sys

for _p in ("/opt/trn_rl_repo",):
    if _p not in sys.path:
        sys.path.insert(0, _p)

import numpy as np

# ---------------- problem constants (hardcoded per spec) ----------------
B, D, H, W = 2, 128, 192, 192
SIGMA_D = 120.0
SIGMA_R = 1.2
A = 2.0 * SIGMA_R * SIGMA_R                 # 2.88
ALPHA = float(np.exp(-1.0 / (2.0 * SIGMA_D * SIGMA_D)))

N_CORES = 8
HPC = H // N_CORES                          # 24 output rows per core
WW = W + 2                                  # W padded with replicate halo
HH = HPC + 2                                # slab rows incl. halo

# tunables
J = 4                                       # polynomial degree for exp(2t/a)
NMOM = J + 2                                # moments G_0..G_{J+1}
CH = 6                                      # output rows per chunk
NCH = HPC // CH
SUBROWS = 2                                 # rows per PSUM sub-chunk (<=512 fp32 bank)
NSUB = CH // SUBROWS
MM_DTYPE = "float32r"                       # conv matmul dtype


def _fit_poly(deg):
    # least-squares fit of exp(2t/A) at Chebyshev nodes on [0,1]
    t = (np.cos(np.pi * (np.arange(4000) + 0.5) / 4000) + 1.0) / 2.0
    y = np.exp(2.0 * t / A)
    V = np.vander(t, deg + 1, increasing=True)
    p, *_ = np.linalg.lstsq(V, y, rcond=None)
    return [float(c) for c in p]


PCOEF = _fit_poly(J)


def _band_matrices():
    """D-axis conv band matrix (replicate-edge corners) x 3 spatial scales."""
    b0 = np.zeros((128, 128), np.float64)
    for i in range(128):
        b0[i, i] = 1.0
        if i > 0:
            b0[i - 1, i] = ALPHA
        if i < 127:
            b0[i + 1, i] = ALPHA
    b0[0, 0] += ALPHA
    b0[127, 127] += ALPHA
    bands = np.concatenate(
        [b0, ALPHA * b0, (ALPHA * ALPHA) * b0], axis=1
    )  # [128, 384]
    return bands.astype(np.float32)


_COMPILED = None


def _build():
    import concourse.bacc as bacc
    import concourse.mybir as mybir
    import concourse.tile as tile

    f32 = mybir.dt.float32
    mmdt = getattr(mybir.dt, MM_DTYPE)
    AF = mybir.ActivationFunctionType
    OP = mybir.AluOpType

    nc = bacc.Bacc("TRN2", target_bir_lowering=False, debug=False)
    vol = nc.dram_tensor("vol", [B, D, HH, WW], f32, kind="ExternalInput")
    bands = nc.dram_tensor("bands", [128, 3 * 128], f32, kind="ExternalInput")
    out = nc.dram_tensor("out", [B, D, HPC, W], f32, kind="ExternalOutput")

    HR = CH + 2                     # halo rows per chunk
    FHALO = HR * WW                 # free size of halo-extent (phi) tiles
    FOUT = CH * W                   # free size of output-extent tiles
    FSUB = SUBROWS * W              # free size of one PSUM sub-chunk

    with tile.TileContext(nc) as tc:
        with tc.tile_pool(name="const", bufs=1) as cpool, \
             tc.tile_pool(name="slab", bufs=1) as spool, \
             tc.tile_pool(name="sbuf", bufs=2) as pool, \
             tc.tile_pool(name="gpool", bufs=2) as gpool, \
             tc.tile_pool(name="hpool", bufs=1) as hpool, \
             tc.tile_pool(name="psum", bufs=8, space="PSUM") as psum:

            bf = cpool.tile([128, 3 * 128], f32, tag="bands_f32")
            nc.sync.dma_start(bf[:, :], bands.ap())
            bmm = cpool.tile([128, 3 * 128], mmdt, tag="bands_mm")
            nc.vector.tensor_copy(bmm[:, :], bf[:, :])
            bmats = [bmm[:, 128 * m:128 * (m + 1)] for m in range(3)]

            # (dh, dw) -> band matrix index by dh^2+dw^2
            offsets = [(dh, dw) for dh in (-1, 0, 1) for dw in (-1, 0, 1)]

            for b in range(B):
                bsl = spool.tile([128, HH * WW], f32, tag="bslab")
                nc.sync.dma_start(bsl[:, :], vol.ap()[b, :, :, :])
                bslv = bsl[:, :].rearrange("p (r w) -> p r w", r=HH)

                for ich in range(NCH):
                    r0 = ich * CH          # first output row of chunk
                    # halo-extent view of this chunk within the b-slab
                    vch = bslv[:, r0:r0 + HR, :]

                    phis = []
                    ph0 = pool.tile([128, FHALO], mmdt, tag="phi0")
                    nc.scalar.activation(ph0[:, :], vch, AF.Square)
                    nc.scalar.activation(ph0[:, :], ph0[:, :], AF.Exp,
                                         scale=-1.0 / A)
                    phis.append(ph0)
                    for j in range(1, NMOM):
                        pj = pool.tile([128, FHALO], mmdt, tag=f"phi{j}",
                                       name=f"phi{j}_{b}_{ich}")
                        nc.vector.tensor_tensor(
                            pj[:, :], phis[-1][:, :], vch, op=OP.mult)
                        phis.append(pj)

                    gt = [gpool.tile([128, FOUT], f32, tag=f"G{j}",
                                     name=f"G{j}_{b}_{ich}")
                          for j in range(NMOM)]
                    for isub in range(NSUB):
                        rr = isub * SUBROWS            # output row in chunk
                        for j in range(NMOM):
                            ps = psum.tile([128, FSUB], f32, tag="ps")
                            pv = phis[j][:, :].rearrange(
                                "p (r w) -> p r w", r=HR)
                            for k, (dh, dw) in enumerate(offsets):
                                m = dh * dh + dw * dw
                                rhs = pv[:, rr + 1 + dh: rr + 1 + dh + SUBROWS,
                                         dw + 1: dw + 1 + W]
                                nc.tensor.matmul(
                                    ps[:, :], bmats[m], rhs,
                                    start=(k == 0), stop=(k == len(offsets) - 1))
                            nc.scalar.copy(
                                gt[j][:, rr * W:(rr + SUBROWS) * W], ps[:, :])

                    # center values c: interior of the chunk
                    cap = bslv[:, r0 + 1:r0 + 1 + CH, 1:W + 1]

                    hn = hpool.tile([128, FOUT], f32, tag="hn")
                    hd = hpool.tile([128, FOUT], f32, tag="hd")
                    tn = hpool.tile([128, FOUT], f32, tag="tn")
                    td = hpool.tile([128, FOUT], f32, tag="td")
                    nc.scalar.mul(hd[:, :], gt[J][:, :], PCOEF[J])
                    nc.scalar.mul(hn[:, :], gt[J + 1][:, :], PCOEF[J])
                    for j in range(J - 1, -1, -1):
                        nc.vector.tensor_tensor(td[:, :], hd[:, :], cap, op=OP.mult)
                        nc.vector.scalar_tensor_tensor(
                            hd[:, :], gt[j][:, :], PCOEF[j], td[:, :],
                            op0=OP.mult, op1=OP.add)
                        nc.vector.tensor_tensor(tn[:, :], hn[:, :], cap, op=OP.mult)
                        nc.vector.scalar_tensor_tensor(
                            hn[:, :], gt[j + 1][:, :], PCOEF[j], tn[:, :],
                            op0=OP.mult, op1=OP.add)

                    # out = hn / hd
                    nc.vector.reciprocal_approx_accurate(
                        out=td[:, :], in_=hd[:, :], scratch=tn[:, :])
                    ot = pool.tile([128, FOUT], f32, tag="ot")
                    nc.vector.tensor_tensor(ot[:, :], hn[:, :], td[:, :], op=OP.mult)

                    nc.sync.dma_start(out.ap()[b, :, r0:r0 + CH, :], ot[:, :])

    nc.compile()
    return nc


def _get_compiled():
    global _COMPILED
    if _COMPILED is None:
        _COMPILED = _build()
    return _COMPILED


def _shard_inputs(volume):
    v = np.asarray(volume)[:, 0]                          # (B, D, H, W)
    vp = np.pad(v, ((0, 0), (0, 0), (1, 1), (1, 1)), mode="edge")
    bands = _band_matrices()
    in_maps = []
    for c in range(N_CORES):
        slab = np.ascontiguousarray(vp[:, :, c * HPC:c * HPC + HH, :])
        in_maps.append({"vol": slab, "bands": bands})
    return in_maps


def _run(volume, trace=False):
    from concourse import bass_utils
    nc = _get_compiled()
    in_maps = _shard_inputs(volume)
    res = bass_utils.run_bass_kernel_spmd(
        nc, in_maps, core_ids=list(range(N_CORES)), trace=trace)
    shards = [res.results[c]["out"] for c in range(N_CORES)]
    full = np.concatenate(shards, axis=2)                 # (B, D, H, W)
    return full[:, None].astype(np.float32), res


def kernel(volume):
    out, _ = _run(volume, trace=False)
    return out


# revision 10
# speedup vs baseline: 1.5089x; 1.2590x over previous
"""3D bilateral filter (window 3, sigma_d=120, sigma_r=1.2) on 8 TRN2 NeuronCores.

Algorithm: factor the range kernel
    exp(-(n-c)^2/a) = phi(n) * phi(c) * exp(2*n*c/a),   phi(x) = exp(-x^2/a)
and approximate exp(2*t/a) on t in [0,1] by a degree-J polynomial
    exp(2t/a) ~= sum_j p_j t^j.
Then with moment fields  phi_j = phi(v) * v^j  and  G_j = conv3x3x3(s, phi_j)
(s = separable spatial Gaussian [alpha,1,alpha] per axis):
    den = phi(c) * sum_j p_j c^j G_j
    num = phi(c) * sum_j p_j c^j G_{j+1}
    out = num / den            (phi(c) cancels)
The 3D conv runs on the Tensor engine: the D-axis (partition dim) conv is a
banded 128x128 matmul (replicate edges folded into the corner entries), and
the 9 (dh,dw) shifts are free-dim AP offsets accumulated in PSUM.

Sharding: 8 cores split H (192 -> 24 rows each) with 1-row halo overlap,
prepared host-side. No cross-core communication.
"""

import sys

for _p in ("/opt/trn_rl_repo",):
    if _p not in sys.path:
        sys.path.insert(0, _p)

import numpy as np

# ---------------- problem constants (hardcoded per spec) ----------------
B, D, H, W = 2, 128, 192, 192
SIGMA_D = 120.0
SIGMA_R = 1.2
A = 2.0 * SIGMA_R * SIGMA_R                 # 2.88
ALPHA = float(np.exp(-1.0 / (2.0 * SIGMA_D * SIGMA_D)))

N_CORES = 8
HPC = H // N_CORES                          # 24 output rows per core
WW = W + 2                                  # W padded with replicate halo
HH = HPC + 2                                # slab rows incl. halo

# tunables
J = 3                                       # polynomial degree for exp(2t/a)
NMOM = J + 2                                # moments G_0..G_{J+1}
CH = 6                                      # output rows per chunk
NCH = HPC // CH
SUBROWS = 2                                 # rows per PSUM sub-chunk (<=512 fp32 bank)
NSUB = CH // SUBROWS
MM_DTYPE = "float32r"                       # conv matmul dtype


def _fit_poly(deg):
    # least-squares fit of exp(2t/A) at Chebyshev nodes on [0,1]
    t = (np.cos(np.pi * (np.arange(4000) + 0.5) / 4000) + 1.0) / 2.0
    y = np.exp(2.0 * t / A)
    V = np.vander(t, deg + 1, increasing=True)
    p, *_ = np.linalg.lstsq(V, y, rcond=None)
    return [float(c) for c in p]


PCOEF = _fit_poly(J)


def _band_matrices():
    """D-axis conv band matrix (replicate-edge corners) x 3 spatial scales."""
    b0 = np.zeros((128, 128), np.float64)
    for i in range(128):
        b0[i, i] = 1.0
        if i > 0:
            b0[i - 1, i] = ALPHA
        if i < 127:
            b0[i + 1, i] = ALPHA
    b0[0, 0] += ALPHA
    b0[127, 127] += ALPHA
    bands = np.concatenate(
        [b0, ALPHA * b0, (ALPHA * ALPHA) * b0], axis=1
    )  # [128, 384]
    return bands.astype(np.float32)


_COMPILED = None


def _build():
    import concourse.bacc as bacc
    import concourse.mybir as mybir
    import concourse.tile as tile

    f32 = mybir.dt.float32
    mmdt = getattr(mybir.dt, MM_DTYPE)
    AF = mybir.ActivationFunctionType
    OP = mybir.AluOpType

    nc = bacc.Bacc("TRN2", target_bir_lowering=False, debug=False)
    vol = nc.dram_tensor("vol", [B, D, HH, WW], f32, kind="ExternalInput")
    bands = nc.dram_tensor("bands", [128, 3 * 128], f32, kind="ExternalInput")
    out = nc.dram_tensor("out", [B, D, HPC, W], f32, kind="ExternalOutput")

    HR = CH + 2                     # halo rows per chunk
    FHALO = HR * WW                 # free size of halo-extent (phi) tiles
    FOUT = CH * W                   # free size of output-extent tiles
    FSUB = SUBROWS * W              # free size of one PSUM sub-chunk

    with tile.TileContext(nc) as tc:
        with tc.tile_pool(name="const", bufs=1) as cpool, \
             tc.tile_pool(name="slab", bufs=2) as spool, \
             tc.tile_pool(name="sbuf", bufs=2) as pool, \
             tc.tile_pool(name="gpool", bufs=2) as gpool, \
             tc.tile_pool(name="hpool", bufs=1) as hpool, \
             tc.tile_pool(name="psum", bufs=8, space="PSUM") as psum:

            bf = cpool.tile([128, 3 * 128], f32, tag="bands_f32")
            nc.sync.dma_start(bf[:, :], bands.ap())
            bmm = cpool.tile([128, 3 * 128], mmdt, tag="bands_mm")
            nc.vector.tensor_copy(bmm[:, :], bf[:, :])
            bmats = [bmm[:, 128 * m:128 * (m + 1)] for m in range(3)]

            # (dh, dw) -> band matrix index by dh^2+dw^2
            offsets = [(dh, dw) for dh in (-1, 0, 1) for dw in (-1, 0, 1)]

            for b in range(B):
                bsl = spool.tile([128, HH * WW], f32, tag="bslab")
                hh2 = HH // 2
                nc.sync.dma_start(bsl[:, :hh2 * WW], vol.ap()[b, :, :hh2, :])
                nc.sync.dma_start(bsl[:, hh2 * WW:], vol.ap()[b, :, hh2:, :])
                bslv = bsl[:, :].rearrange("p (r w) -> p r w", r=HH)

                for ich in range(NCH):
                    r0 = ich * CH          # first output row of chunk
                    # halo-extent view of this chunk within the b-slab
                    vch = bslv[:, r0:r0 + HR, :]

                    phis = []
                    ph0 = pool.tile([128, FHALO], mmdt, tag="phi0")
                    nc.scalar.activation(ph0[:, :], vch, AF.Square)
                    nc.scalar.activation(ph0[:, :], ph0[:, :], AF.Exp,
                                         scale=-1.0 / A)
                    phis.append(ph0)
                    for j in range(1, NMOM):
                        pj = pool.tile([128, FHALO], mmdt, tag=f"phi{j}",
                                       name=f"phi{j}_{b}_{ich}")
                        nc.vector.tensor_tensor(
                            pj[:, :], phis[-1][:, :], vch, op=OP.mult)
                        phis.append(pj)

                    gt = [gpool.tile([128, FOUT], f32, tag=f"G{j}",
                                     name=f"G{j}_{b}_{ich}")
                          for j in range(NMOM)]
                    for isub in range(NSUB):
                        rr = isub * SUBROWS            # output row in chunk
                        for j in range(NMOM):
                            ps = psum.tile([128, FSUB], f32, tag="ps")
                            pv = phis[j][:, :].rearrange(
                                "p (r w) -> p r w", r=HR)
                            for k, (dh, dw) in enumerate(offsets):
                                m = dh * dh + dw * dw
                                rhs = pv[:, rr + 1 + dh: rr + 1 + dh + SUBROWS,
                                         dw + 1: dw + 1 + W]
                                nc.tensor.matmul(
                                    ps[:, :], bmats[m], rhs,
                                    start=(k == 0), stop=(k == len(offsets) - 1))
                            nc.scalar.copy(
                                gt[j][:, rr * W:(rr + SUBROWS) * W], ps[:, :])

                    # center values c: interior of the chunk
                    cap = bslv[:, r0 + 1:r0 + 1 + CH, 1:W + 1]

                    hn = hpool.tile([128, FOUT], f32, tag="hn")
                    hd = hpool.tile([128, FOUT], f32, tag="hd")
                    tn = hpool.tile([128, FOUT], f32, tag="tn")
                    td = hpool.tile([128, FOUT], f32, tag="td")
                    nc.scalar.mul(hd[:, :], gt[J][:, :], PCOEF[J])
                    nc.scalar.mul(hn[:, :], gt[J + 1][:, :], PCOEF[J])
                    for j in range(J - 1, -1, -1):
                        nc.vector.tensor_tensor(td[:, :], hd[:, :], cap, op=OP.mult)
                        nc.vector.scalar_tensor_tensor(
                            hd[:, :], gt[j][:, :], PCOEF[j], td[:, :],
                            op0=OP.mult, op1=OP.add)
                        nc.vector.tensor_tensor(tn[:, :], hn[:, :], cap, op=OP.mult)
                        nc.vector.scalar_tensor_tensor(
                            hn[:, :], gt[j + 1][:, :], PCOEF[j], tn[:, :],
                            op0=OP.mult, op1=OP.add)

                    # out = hn / hd  (hd in [14, 28] — approx recip is safe)
                    nc.vector.reciprocal_approx_fast(out=td[:, :], in_=hd[:, :])
                    ot = pool.tile([128, FOUT], f32, tag="ot")
                    nc.vector.tensor_tensor(ot[:, :], hn[:, :], td[:, :], op=OP.mult)

                    nc.sync.dma_start(out.ap()[b, :, r0:r0 + CH, :], ot[:, :])

    nc.compile()
    return nc


def _get_compiled():
    global _COMPILED
    if _COMPILED is None:
        _COMPILED = _build()
    return _COMPILED


def _shard_inputs(volume):
    v = np.asarray(volume)[:, 0]                          # (B, D, H, W)
    vp = np.pad(v, ((0, 0), (0, 0), (1, 1), (1, 1)), mode="edge")
    bands = _band_matrices()
    in_maps = []
    for c in range(N_CORES):
        slab = np.ascontiguousarray(vp[:, :, c * HPC:c * HPC + HH, :])
        in_maps.append({"vol": slab, "bands": bands})
    return in_maps


def _run(volume, trace=False):
    from concourse import bass_utils
    nc = _get_compiled()
    in_maps = _shard_inputs(volume)
    res = bass_utils.run_bass_kernel_spmd(
        nc, in_maps, core_ids=list(range(N_CORES)), trace=trace)
    shards = [res.results[c]["out"] for c in range(N_CORES)]
    full = np.concatenate(shards, axis=2)                 # (B, D, H, W)
    return full[:, None].astype(np.float32), res


def kernel(volume):
    out, _ = _run(volume, trace=False)
    return out


# revision 12
# speedup vs baseline: 1.7142x; 1.1361x over previous
"""3D bilateral filter (window 3, sigma_d=120, sigma_r=1.2) on 8 TRN2 NeuronCores.

Algorithm: factor the range kernel
    exp(-(n-c)^2/a) = phi(n) * phi(c) * exp(2*n*c/a),   phi(x) = exp(-x^2/a)
and approximate exp(2*t/a) on t in [0,1] by a degree-J polynomial
    exp(2t/a) ~= sum_j p_j t^j.
Then with moment fields  phi_j = phi(v) * v^j  and  G_j = conv3x3x3(s, phi_j)
(s = separable spatial Gaussian [alpha,1,alpha] per axis):
    den = phi(c) * sum_j p_j c^j G_j
    num = phi(c) * sum_j p_j c^j G_{j+1}
    out = num / den            (phi(c) cancels)
The 3D conv runs on the Tensor engine: the D-axis (partition dim) conv is a
banded 128x128 matmul (replicate edges folded into the corner entries), and
the 9 (dh,dw) shifts are free-dim AP offsets accumulated in PSUM.

Sharding: 8 cores split H (192 -> 24 rows each) with 1-row halo overlap,
prepared host-side. No cross-core communication.
"""

import sys

for _p in ("/opt/trn_rl_repo",):
    if _p not in sys.path:
        sys.path.insert(0, _p)

import numpy as np

# ---------------- problem constants (hardcoded per spec) ----------------
B, D, H, W = 2, 128, 192, 192
SIGMA_D = 120.0
SIGMA_R = 1.2
A = 2.0 * SIGMA_R * SIGMA_R                 # 2.88
ALPHA = float(np.exp(-1.0 / (2.0 * SIGMA_D * SIGMA_D)))

N_CORES = 8
HPC = H // N_CORES                          # 24 output rows per core
WW = W + 2                                  # W padded with replicate halo
HH = HPC + 2                                # slab rows incl. halo

# tunables
J = 3                                       # polynomial degree for exp(2t/a)
NMOM = J + 2                                # moments G_0..G_{J+1}
CH = 6                                      # output rows per chunk
NCH = HPC // CH
SUBROWS = 2                                 # rows per PSUM sub-chunk (<=512 fp32 bank)
NSUB = CH // SUBROWS
MM_DTYPE = "float32r"                       # conv matmul dtype


def _fit_poly(deg):
    # least-squares fit of exp(2t/A) at Chebyshev nodes on [0,1]
    t = (np.cos(np.pi * (np.arange(4000) + 0.5) / 4000) + 1.0) / 2.0
    y = np.exp(2.0 * t / A)
    V = np.vander(t, deg + 1, increasing=True)
    p, *_ = np.linalg.lstsq(V, y, rcond=None)
    return [float(c) for c in p]


PCOEF = _fit_poly(J)


def _band_matrices():
    """D-axis conv band matrix (replicate-edge corners) x 3 spatial scales."""
    b0 = np.zeros((128, 128), np.float64)
    for i in range(128):
        b0[i, i] = 1.0
        if i > 0:
            b0[i - 1, i] = ALPHA
        if i < 127:
            b0[i + 1, i] = ALPHA
    b0[0, 0] += ALPHA
    b0[127, 127] += ALPHA
    bands = np.concatenate(
        [b0, ALPHA * b0, (ALPHA * ALPHA) * b0], axis=1
    )  # [128, 384]
    return bands.astype(np.float32)


_COMPILED = None


def _build():
    import concourse.bacc as bacc
    import concourse.mybir as mybir
    import concourse.tile as tile

    f32 = mybir.dt.float32
    mmdt = getattr(mybir.dt, MM_DTYPE)
    AF = mybir.ActivationFunctionType
    OP = mybir.AluOpType

    nc = bacc.Bacc("TRN2", target_bir_lowering=False, debug=False)
    vol = nc.dram_tensor("vol", [B, D, HH, WW], f32, kind="ExternalInput")
    bands = nc.dram_tensor("bands", [128, 3 * 128], f32, kind="ExternalInput")
    out = nc.dram_tensor("out", [B, D, HPC, W], f32, kind="ExternalOutput")

    HR = CH + 2                     # halo rows per chunk
    FHALO = HR * WW                 # free size of halo-extent (phi) tiles
    FOUT = CH * W                   # free size of output-extent tiles
    FSUB = SUBROWS * W              # free size of one PSUM sub-chunk

    with tile.TileContext(nc) as tc:
        with tc.tile_pool(name="const", bufs=1) as cpool, \
             tc.tile_pool(name="slab", bufs=2) as spool, \
             tc.tile_pool(name="sbuf", bufs=2) as pool, \
             tc.tile_pool(name="gpool", bufs=2) as gpool, \
             tc.tile_pool(name="hpool", bufs=1) as hpool, \
             tc.tile_pool(name="psum", bufs=8, space="PSUM") as psum:

            bf = cpool.tile([128, 3 * 128], f32, tag="bands_f32")
            nc.sync.dma_start(bf[:, :], bands.ap())
            bmm = cpool.tile([128, 3 * 128], mmdt, tag="bands_mm")
            nc.vector.tensor_copy(bmm[:, :], bf[:, :])
            bmats = [bmm[:, 128 * m:128 * (m + 1)] for m in range(3)]

            # (dh, dw) -> band matrix index by dh^2+dw^2
            offsets = [(dh, dw) for dh in (-1, 0, 1) for dw in (-1, 0, 1)]

            def emit_horner(gt, cap, b, r0):
                hn = hpool.tile([128, FOUT], f32, tag="hn")
                hd = hpool.tile([128, FOUT], f32, tag="hd")
                tn = hpool.tile([128, FOUT], f32, tag="tn")
                td = hpool.tile([128, FOUT], f32, tag="td")
                nc.scalar.mul(hd[:, :], gt[J][:, :], PCOEF[J])
                nc.scalar.mul(hn[:, :], gt[J + 1][:, :], PCOEF[J])
                for j in range(J - 1, -1, -1):
                    nc.vector.tensor_tensor(td[:, :], hd[:, :], cap, op=OP.mult)
                    nc.vector.scalar_tensor_tensor(
                        hd[:, :], gt[j][:, :], PCOEF[j], td[:, :],
                        op0=OP.mult, op1=OP.add)
                    nc.vector.tensor_tensor(tn[:, :], hn[:, :], cap, op=OP.mult)
                    nc.vector.scalar_tensor_tensor(
                        hn[:, :], gt[j + 1][:, :], PCOEF[j], tn[:, :],
                        op0=OP.mult, op1=OP.add)
                # out = hn / hd  (hd in [14, 28] — approx recip is safe)
                nc.vector.reciprocal_approx_fast(out=td[:, :], in_=hd[:, :])
                ot = pool.tile([128, FOUT], f32, tag="ot")
                nc.vector.tensor_tensor(ot[:, :], hn[:, :], td[:, :], op=OP.mult)
                nc.sync.dma_start(out.ap()[b, :, r0:r0 + CH, :], ot[:, :])

            pending = None
            for b in range(B):
                bsl = spool.tile([128, HH * WW], f32, tag="bslab")
                hh2 = HH // 2
                nc.sync.dma_start(bsl[:, :hh2 * WW], vol.ap()[b, :, :hh2, :])
                nc.sync.dma_start(bsl[:, hh2 * WW:], vol.ap()[b, :, hh2:, :])
                bslv = bsl[:, :].rearrange("p (r w) -> p r w", r=HH)

                for ich in range(NCH):
                    r0 = ich * CH          # first output row of chunk
                    # halo-extent view of this chunk within the b-slab
                    vch = bslv[:, r0:r0 + HR, :]

                    phis = []
                    ph0 = pool.tile([128, FHALO], mmdt, tag="phi0")
                    nc.scalar.activation(ph0[:, :], vch, AF.Square)
                    nc.scalar.activation(ph0[:, :], ph0[:, :], AF.Exp,
                                         scale=-1.0 / A)
                    phis.append(ph0)
                    for j in range(1, NMOM):
                        pj = pool.tile([128, FHALO], mmdt, tag=f"phi{j}",
                                       name=f"phi{j}_{b}_{ich}")
                        nc.vector.tensor_tensor(
                            pj[:, :], phis[-1][:, :], vch, op=OP.mult)
                        phis.append(pj)

                    gt = [gpool.tile([128, FOUT], f32, tag=f"G{j}",
                                     name=f"G{j}_{b}_{ich}")
                          for j in range(NMOM)]
                    for isub in range(NSUB):
                        rr = isub * SUBROWS            # output row in chunk
                        for j in range(NMOM):
                            ps = psum.tile([128, FSUB], f32, tag="ps")
                            pv = phis[j][:, :].rearrange(
                                "p (r w) -> p r w", r=HR)
                            for k, (dh, dw) in enumerate(offsets):
                                m = dh * dh + dw * dw
                                rhs = pv[:, rr + 1 + dh: rr + 1 + dh + SUBROWS,
                                         dw + 1: dw + 1 + W]
                                nc.tensor.matmul(
                                    ps[:, :], bmats[m], rhs,
                                    start=(k == 0), stop=(k == len(offsets) - 1))
                            nc.scalar.copy(
                                gt[j][:, rr * W:(rr + SUBROWS) * W], ps[:, :])

                    # software pipeline: emit previous chunk's recombination
                    # AFTER this chunk's phi-chain + convs, so the PE never
                    # waits on the Vector engine's Horner phase.
                    if pending is not None:
                        emit_horner(*pending)
                    cap = bslv[:, r0 + 1:r0 + 1 + CH, 1:W + 1]
                    pending = (gt, cap, b, r0)

            emit_horner(*pending)

    nc.compile()
    return nc


def _get_compiled():
    global _COMPILED
    if _COMPILED is None:
        _COMPILED = _build()
    return _COMPILED


def _shard_inputs(volume):
    v = np.asarray(volume)[:, 0]                          # (B, D, H, W)
    vp = np.pad(v, ((0, 0), (0, 0), (1, 1), (1, 1)), mode="edge")
    bands = _band_matrices()
    in_maps = []
    for c in range(N_CORES):
        slab = np.ascontiguousarray(vp[:, :, c * HPC:c * HPC + HH, :])
        in_maps.append({"vol": slab, "bands": bands})
    return in_maps


def _run(volume, trace=False):
    from concourse import bass_utils
    nc = _get_compiled()
    in_maps = _shard_inputs(volume)
    res = bass_utils.run_bass_kernel_spmd(
        nc, in_maps, core_ids=list(range(N_CORES)), trace=trace)
    shards = [res.results[c]["out"] for c in range(N_CORES)]
    full = np.concatenate(shards, axis=2)                 # (B, D, H, W)
    return full[:, None].astype(np.float32), res


def kernel(volume):
    out, _ = _run(volume, trace=False)
    return out
